# revision 11
# baseline (speedup 1.0000x reference)
"""Causal self-attention (sparse column mask) on 8 Trainium2 NeuronCores.

Problem: B=8, T=1024, C=512, 8 heads (hd=64).
  q/k/v = x @ W{q,k,v}.T + b;  att = softmax(mask(q k^T / 8));  y = att v
  out = y @ Wp.T + bp
Mask: causal lower-triangle, minus every column j with j % 25 == 24.

Sharding: pure data-parallel over batch — core b computes batch element b.

Per-core kernel design (all matmul operands fp16, PSUM accumulation f32):
  - Host pre-transposes x[b] -> xT [C, T] and all weights -> W^T [c_in, c_out],
    so every on-chip matmul has its contraction dim on partitions.
  - Projections produce q^T, k^T [C, T] (heads = partition blocks of 64) and
    v [T, C]. q bias is added during PSUM evacuation (DVE tensor_scalar,
    per-partition); k bias is dropped (softmax shift invariance); v bias is
    folded into the output bias on host (bp' = Wp @ bv + bp, sent broadcast).
  - Attention, phase-separated per query chunk ic (512 wide) so the PE array
    stays in one tiling mode per phase (mode switches drain the PE):
      QK phase (64x128 row-tiled): per head-pair p, per key tile J: two K=64
      matmuls (tile_position (0,0)/(64,0)) -> S^T in 2 PSUM banks; one ACT
      exp call over both (scale=1/8, per-partition bias -30 on j%25==24
      columns) -> fp16 SBUF; causal diagonal zeroed by one fp16 multiply with
      a broadcast lower-triangle tile on DVE.
      AV phase (128x64 col-tiled): per pair, accumulate y'^T and the
      replicated denominators (ones-weight matmuls) over J; then rden via
      approx reciprocal and one tensor_tensor multiply PSUM->SBUF fp16.
  - Output projection consumes y_norm^T directly; bias added during the DVE
    evacuation (tensor_tensor add with a host-broadcast bias tile).
"""

import numpy as np

B, T, C = 8, 1024, 512
H = 8
HD = C // H
P = 128
JD = 25  # joined dim; column j masked when j % 25 == 24
N_CORES = 8
NEG = -30.0  # added post-scale; exp(-30) flushes to 0 in fp16

_CACHE = {}


def _build():
    import concourse.bass as bass
    import concourse.mybir as mybir
    import concourse.tile as tile
    from concourse import bacc

    f16 = mybir.dt.float16
    f32 = mybir.dt.float32
    AF = mybir.ActivationFunctionType
    ALU = mybir.AluOpType

    nc = bacc.Bacc("TRN2", target_bir_lowering=False, debug=False)

    xT = nc.dram_tensor("xT", [C, T], f16, kind="ExternalInput").ap()
    wqT = nc.dram_tensor("wqT", [C, C], f16, kind="ExternalInput").ap()
    wkT = nc.dram_tensor("wkT", [C, C], f16, kind="ExternalInput").ap()
    wvT = nc.dram_tensor("wvT", [C, C], f16, kind="ExternalInput").ap()
    wpT = nc.dram_tensor("wpT", [C, C], f16, kind="ExternalInput").ap()
    bq = nc.dram_tensor("bq", [P, C // P], f32, kind="ExternalInput").ap()
    bppb = nc.dram_tensor("bppb", [P, C], f32, kind="ExternalInput").ap()
    ones64 = nc.dram_tensor("ones64", [P, HD], f16, kind="ExternalInput").ap()
    tri = nc.dram_tensor("tri", [P, P], f16, kind="ExternalInput").ap()
    cmask = nc.dram_tensor("cmask", [P, T // P], f32, kind="ExternalInput").ap()
    out = nc.dram_tensor("out", [T, C], f32, kind="ExternalOutput").ap()

    KT = C // P  # 4 c_in tiles
    MT = C // P  # 4 c_out tiles (= head pairs)
    RT = T // P  # 8 t tiles

    with tile.TileContext(nc) as tc:
        with (
            tc.tile_pool(name="const", bufs=1) as const,
            tc.tile_pool(name="persist", bufs=1) as persist,
            tc.tile_pool(name="es", bufs=16) as es_pool,
            tc.tile_pool(name="rden", bufs=2) as rden_pool,
            tc.tile_pool(name="ot", bufs=3) as ot_pool,
            tc.tile_pool(name="pbig", bufs=2, space="PSUM") as pbig,
            tc.tile_pool(name="psmall", bufs=4, space="PSUM") as psmall,
        ):
            # ---- consolidated input loads (big DMAs, two queues) ----
            def load(shape, dtype, src, tag, eng):
                t = const.tile(shape, dtype, name=tag, tag=tag)
                eng.dma_start(out=t, in_=src)
                return t

            r3 = lambda a: a.rearrange("(a p) n -> p a n", p=P)  # noqa: E731
            # first-needed data in small chunks so the first matmuls start early
            xT_lo = load([P, 2, T], f16, r3(xT)[:, 0:2, :], "xTlo", nc.sync)
            wq_lo = load([P, 2, C], f16, r3(wqT)[:, 0:2, :], "wqlo", nc.scalar)
            xT_hi = load([P, 2, T], f16, r3(xT)[:, 2:4, :], "xThi", nc.sync)
            wq_hi = load([P, 2, C], f16, r3(wqT)[:, 2:4, :], "wqhi", nc.scalar)
            wk_a = load([P, KT, C], f16, r3(wkT), "wk", nc.scalar)
            wv_a = load([P, KT, C], f16, r3(wvT), "wv", nc.sync)
            wp_a = load([P, KT, C], f16, r3(wpT), "wp", nc.scalar)
            bq_s = load([P, C // P], f32, bq, "bq", nc.scalar)
            bppb_s = load([P, C], f32, bppb, "bppb", nc.sync)
            ones64_s = load([P, HD], f16, ones64, "ones64", nc.scalar)
            tri_s = load([P, P], f16, tri, "tri", nc.sync)
            cmask_s = load([P, T // P], f32, cmask, "cmask", nc.sync)

            def xt(k):
                return xT_lo[:, k, :] if k < 2 else xT_hi[:, k - 2, :]

            def wq(k):
                return wq_lo[:, k, :] if k < 2 else wq_hi[:, k - 2, :]

            qT_t = [persist.tile([P, T], f16, name=f"qT{m}", tag=f"qT{m}") for m in range(MT)]
            kT_t = [persist.tile([P, T], f16, name=f"kT{m}", tag=f"kT{m}") for m in range(MT)]
            v_t = [persist.tile([P, C], f16, name=f"v{r}", tag=f"v{r}") for r in range(RT)]
            yn_t = [persist.tile([P, T], f16, name=f"yn{m}", tag=f"yn{m}") for m in range(MT)]

            # broadcast lower-triangle tile across both heads of an es tile
            tri_b = bass.AP(
                tensor=tri_s.tensor,
                offset=tri_s.offset,
                ap=[list(tri_s.ap[0]), [0, 2], list(tri_s.ap[1])],
            )

            # ---- emission helpers ----
            def proj_qk(m):
                for which, dst, biased in (("q", qT_t[m], True), ("k", kT_t[m], False)):
                    ps = pbig.tile([P, T], f32, name="psqk", tag="pbig")
                    for half in range(2):
                        o = ps[:, 512 * half : 512 * (half + 1)]
                        for k in range(KT):
                            w_ap = (
                                wq(k)[:, P * m : P * (m + 1)]
                                if which == "q"
                                else wk_a[:, k, P * m : P * (m + 1)]
                            )
                            nc.tensor.matmul(
                                o,
                                lhsT=w_ap,
                                rhs=xt(k)[:, 512 * half : 512 * (half + 1)],
                                start=(k == 0),
                                stop=(k == KT - 1),
                            )
                    if biased:
                        nc.vector.tensor_scalar_add(dst, ps, bq_s[:, m : m + 1])
                    else:
                        nc.vector.tensor_copy(dst, ps)

            def proj_v(r0, r1):
                for r in range(r0, r1):
                    ps = psmall.tile([P, C], f32, name="pv", tag="sm")
                    for k in range(KT):
                        nc.tensor.matmul(
                            ps,
                            lhsT=xt(k)[:, P * r : P * (r + 1)],
                            rhs=wv_a[:, k, :],
                            start=(k == 0),
                            stop=(k == KT - 1),
                        )
                    nc.scalar.activation(v_t[r], ps, AF.Copy)

            es_t = {}

            def qk_phase(ic, p):
                for J in range(4 * (ic + 1)):
                    i0 = max(512 * ic, P * J)
                    w = 512 * (ic + 1) - i0
                    st = pbig.tile([P, 2, 512], f32, name="st", tag="pbig")
                    for h in range(2):
                        nc.tensor.matmul(
                            st[:, h, :w],
                            lhsT=kT_t[p][64 * h : 64 * (h + 1), P * J : P * (J + 1)],
                            rhs=qT_t[p][64 * h : 64 * (h + 1), i0 : i0 + w],
                            start=True,
                            stop=True,
                            tile_position=(64 * h, 0),
                        )
                    es = es_pool.tile([P, 2, 512], f16, name="es", tag="es")
                    es_t[(ic, p, J)] = es
                    nc.scalar.activation(
                        es[:, :, :w],
                        st[:, :, :w],
                        AF.Exp,
                        bias=cmask_s[:, J : J + 1],
                        scale=0.125,
                    )
                    if P * J >= 512 * ic:  # diagonal: zero the causal triangle
                        nc.vector.tensor_tensor(
                            out=es[:, :, :P], in0=es[:, :, :P], in1=tri_b, op=ALU.mult
                        )

            def av_phase(ic, p):
                av = psmall.tile([P, 512], f32, name="av", tag="sm")
                den = psmall.tile([P, 512], f32, name="den", tag="sm")
                nJ = 4 * (ic + 1)
                for J in range(nJ):
                    i0 = max(512 * ic, P * J)
                    w = 512 * (ic + 1) - i0
                    io = i0 - 512 * ic
                    first, last = J == 0, J == nJ - 1
                    es = es_t.pop((ic, p, J))
                    for h in range(2):
                        nc.tensor.matmul(
                            av[64 * h : 64 * (h + 1), io : io + w],
                            lhsT=v_t[J][:, P * p + 64 * h : P * p + 64 * (h + 1)],
                            rhs=es[:, h, :w],
                            start=first,
                            stop=last,
                            tile_position=(0, 64 * h),
                        )
                        nc.tensor.matmul(
                            den[64 * h : 64 * (h + 1), io : io + w],
                            lhsT=ones64_s,
                            rhs=es[:, h, :w],
                            start=first,
                            stop=last,
                            tile_position=(0, 64 * h),
                        )
                rden = rden_pool.tile([P, 512], f32, name="rden", tag="rden")
                nc.vector.reciprocal_approx_fast(out=rden, in_=den)
                nc.vector.tensor_mul(yn_t[p][:, 512 * ic : 512 * (ic + 1)], av, rden)

            def outproj(r0, r1):
                for r in range(r0, r1):
                    po = psmall.tile([P, C], f32, name="po", tag="sm")
                    for m in range(MT):
                        nc.tensor.matmul(
                            po,
                            lhsT=yn_t[m][:, P * r : P * (r + 1)],
                            rhs=wp_a[:, m, :],
                            start=(m == 0),
                            stop=(m == MT - 1),
                        )
                    ot = ot_pool.tile([P, C], f32, name="ot", tag="ot")
                    nc.vector.tensor_tensor(out=ot, in0=po, in1=bppb_s, op=ALU.add)
                    nc.sync.dma_start(out=out[P * r : P * (r + 1), :], in_=ot)

            # ---- emission schedule: weave attention into projections so the
            # ACT exp pipeline starts early and never starves ----
            proj_qk(0)
            qk_phase(0, 0)
            proj_qk(1)
            proj_v(0, 4)
            qk_phase(0, 1)
            av_phase(0, 0)
            proj_qk(2)
            proj_v(4, 8)
            qk_phase(0, 2)
            av_phase(0, 1)
            proj_qk(3)
            qk_phase(0, 3)
            av_phase(0, 2)
            av_phase(0, 3)
            qk_phase(1, 0)
            outproj(0, 4)
            qk_phase(1, 1)
            av_phase(1, 0)
            qk_phase(1, 2)
            av_phase(1, 1)
            qk_phase(1, 3)
            av_phase(1, 2)
            av_phase(1, 3)
            outproj(4, 8)

    nc.compile()
    return nc


def _prep_inputs(x, Wq, bq, Wk, bk, Wv, bv, Wp, bp):
    """Host-side prep: transposes, bias folding, mask tables. Returns in_maps."""
    f16 = np.float16
    wqT = np.ascontiguousarray(Wq.T).astype(f16)
    wkT = np.ascontiguousarray(Wk.T).astype(f16)
    wvT = np.ascontiguousarray(Wv.T).astype(f16)
    wpT = np.ascontiguousarray(Wp.T).astype(f16)
    bq_pp = np.ascontiguousarray(bq.astype(np.float32).reshape(C // P, P).T)
    # v bias folds into output bias: out = (y' + bv) @ Wp.T + bp
    bpp = (
        Wp.astype(np.float64) @ bv.astype(np.float64) + bp.astype(np.float64)
    ).astype(np.float32)
    bppb = np.broadcast_to(bpp[None, :], (P, C)).copy()
    ones64 = np.ones((P, HD), dtype=f16)
    tri = (np.arange(P)[:, None] <= np.arange(P)[None, :]).astype(f16)  # keep j<=i
    j_idx = np.arange(P)[:, None] + P * np.arange(T // P)[None, :]
    cmask = np.where(j_idx % JD == JD - 1, np.float32(NEG), np.float32(0.0)).astype(
        np.float32
    )

    shared = {
        "wqT": wqT,
        "wkT": wkT,
        "wvT": wvT,
        "wpT": wpT,
        "bq": bq_pp,
        "bppb": bppb,
        "ones64": ones64,
        "tri": tri,
        "cmask": cmask,
    }
    in_maps = []
    for b in range(N_CORES):
        m = dict(shared)
        m["xT"] = np.ascontiguousarray(x[b].T).astype(f16)
        in_maps.append(m)
    return in_maps


def kernel(x, Wq, bq, Wk, bk, Wv, bv, Wp, bp):
    from concourse import bass_utils

    x = np.asarray(x, dtype=np.float32)
    if "nc" not in _CACHE:
        _CACHE["nc"] = _build()
    nc = _CACHE["nc"]
    in_maps = _prep_inputs(
        x,
        np.asarray(Wq, np.float32),
        np.asarray(bq, np.float32),
        np.asarray(Wk, np.float32),
        np.asarray(bk, np.float32),
        np.asarray(Wv, np.float32),
        np.asarray(bv, np.float32),
        np.asarray(Wp, np.float32),
        np.asarray(bp, np.float32),
    )
    res = bass_utils.run_bass_kernel_spmd(nc, in_maps, core_ids=list(range(N_CORES)))
    return np.stack([res.results[b]["out"] for b in range(N_CORES)], axis=0)


# revision 13
# speedup vs baseline: 1.0264x; 1.0264x over previous
"""Causal self-attention (sparse column mask) on 8 Trainium2 NeuronCores.

Problem: B=8, T=1024, C=512, 8 heads (hd=64).
  q/k/v = x @ W{q,k,v}.T + b;  att = softmax(mask(q k^T / 8));  y = att v
  out = y @ Wp.T + bp
Mask: causal lower-triangle, minus every column j with j % 25 == 24.

Sharding: pure data-parallel over batch — core b computes batch element b.

Per-core kernel design (all matmul operands fp16, PSUM accumulation f32):
  - Host pre-transposes x[b] -> xT [C, T] and all weights -> W^T [c_in, c_out],
    so every on-chip matmul has its contraction dim on partitions.
  - Projections produce q^T, k^T [C, T] (heads = partition blocks of 64) and
    v [T, C]. q bias is added during PSUM evacuation (DVE tensor_scalar,
    per-partition); k bias is dropped (softmax shift invariance); v bias is
    folded into the output bias on host (bp' = Wp @ bv + bp, sent broadcast).
  - Attention, phase-separated per query chunk ic (512 wide) so the PE array
    stays in one tiling mode per phase (mode switches drain the PE):
      QK phase (64x128 row-tiled): per head-pair p, per key tile J: two K=64
      matmuls (tile_position (0,0)/(64,0)) -> S^T in 2 PSUM banks; one ACT
      exp call over both (scale=1/8, per-partition bias -30 on j%25==24
      columns) -> fp16 SBUF; causal diagonal zeroed by one fp16 multiply with
      a broadcast lower-triangle tile on DVE.
      AV phase (128x64 col-tiled): per pair, accumulate y'^T and the
      replicated denominators (ones-weight matmuls) over J; then rden via
      approx reciprocal and one tensor_tensor multiply PSUM->SBUF fp16.
  - Output projection consumes y_norm^T directly; bias added during the DVE
    evacuation (tensor_tensor add with a host-broadcast bias tile).
"""

import numpy as np

B, T, C = 8, 1024, 512
H = 8
HD = C // H
P = 128
JD = 25  # joined dim; column j masked when j % 25 == 24
N_CORES = 8
NEG = -30.0  # added post-scale; exp(-30) flushes to 0 in fp16

_CACHE = {}


def _build():
    import concourse.bass as bass
    import concourse.mybir as mybir
    import concourse.tile as tile
    from concourse import bacc

    f16 = mybir.dt.float16
    f32 = mybir.dt.float32
    AF = mybir.ActivationFunctionType
    ALU = mybir.AluOpType

    nc = bacc.Bacc("TRN2", target_bir_lowering=False, debug=False)

    xT = nc.dram_tensor("xT", [C, T], f16, kind="ExternalInput").ap()
    wqT = nc.dram_tensor("wqT", [C, C], f16, kind="ExternalInput").ap()
    wkT = nc.dram_tensor("wkT", [C, C], f16, kind="ExternalInput").ap()
    wvT = nc.dram_tensor("wvT", [C, C], f16, kind="ExternalInput").ap()
    wpT = nc.dram_tensor("wpT", [C, C], f16, kind="ExternalInput").ap()
    bq = nc.dram_tensor("bq", [P, C // P], f32, kind="ExternalInput").ap()
    bppb = nc.dram_tensor("bppb", [P, C], f32, kind="ExternalInput").ap()
    ones64 = nc.dram_tensor("ones64", [P, HD], f16, kind="ExternalInput").ap()
    tri = nc.dram_tensor("tri", [P, P], f16, kind="ExternalInput").ap()
    cmask = nc.dram_tensor("cmask", [P, T // P], f32, kind="ExternalInput").ap()
    out = nc.dram_tensor("out", [T, C], f32, kind="ExternalOutput").ap()

    KT = C // P  # 4 c_in tiles
    MT = C // P  # 4 c_out tiles (= head pairs)
    RT = T // P  # 8 t tiles

    with tile.TileContext(nc) as tc:
        with (
            tc.tile_pool(name="const", bufs=1) as const,
            tc.tile_pool(name="persist", bufs=1) as persist,
            tc.tile_pool(name="es", bufs=24) as es_pool,
            tc.tile_pool(name="rden", bufs=2) as rden_pool,
            tc.tile_pool(name="ot", bufs=3) as ot_pool,
            tc.tile_pool(name="pbig", bufs=2, space="PSUM") as pbig,
            tc.tile_pool(name="psmall", bufs=4, space="PSUM") as psmall,
        ):
            # ---- consolidated input loads (big DMAs, two queues) ----
            def load(shape, dtype, src, tag, eng):
                t = const.tile(shape, dtype, name=tag, tag=tag)
                eng.dma_start(out=t, in_=src)
                return t

            r3 = lambda a: a.rearrange("(a p) n -> p a n", p=P)  # noqa: E731
            # first-needed data in small chunks so the first matmuls start early
            xT_lo = load([P, 2, T], f16, r3(xT)[:, 0:2, :], "xTlo", nc.sync)
            wq_lo = load([P, 2, C], f16, r3(wqT)[:, 0:2, :], "wqlo", nc.scalar)
            xT_hi = load([P, 2, T], f16, r3(xT)[:, 2:4, :], "xThi", nc.sync)
            wq_hi = load([P, 2, C], f16, r3(wqT)[:, 2:4, :], "wqhi", nc.scalar)
            wk_a = load([P, KT, C], f16, r3(wkT), "wk", nc.scalar)
            tri_s = load([P, P], f16, tri, "tri", nc.sync)
            cmask_s = load([P, T // P], f32, cmask, "cmask", nc.sync)
            wv_a = load([P, KT, C], f16, r3(wvT), "wv", nc.sync)
            bq_s = load([P, C // P], f32, bq, "bq", nc.scalar)
            ones64_s = load([P, HD], f16, ones64, "ones64", nc.scalar)
            wp_a = load([P, KT, C], f16, r3(wpT), "wp", nc.scalar)
            bppb_s = load([P, C], f32, bppb, "bppb", nc.sync)

            def xt(k):
                return xT_lo[:, k, :] if k < 2 else xT_hi[:, k - 2, :]

            def wq(k):
                return wq_lo[:, k, :] if k < 2 else wq_hi[:, k - 2, :]

            qT_t = [persist.tile([P, T], f16, name=f"qT{m}", tag=f"qT{m}") for m in range(MT)]
            kT_t = [persist.tile([P, T], f16, name=f"kT{m}", tag=f"kT{m}") for m in range(MT)]
            v_t = [persist.tile([P, C], f16, name=f"v{r}", tag=f"v{r}") for r in range(RT)]
            yn_t = [persist.tile([P, T], f16, name=f"yn{m}", tag=f"yn{m}") for m in range(MT)]

            # broadcast lower-triangle tile across both heads of an es tile
            tri_b = bass.AP(
                tensor=tri_s.tensor,
                offset=tri_s.offset,
                ap=[list(tri_s.ap[0]), [0, 2], list(tri_s.ap[1])],
            )

            # ---- emission helpers ----
            def proj_qk(m):
                for which, dst, biased in (("q", qT_t[m], True), ("k", kT_t[m], False)):
                    ps = pbig.tile([P, T], f32, name="psqk", tag="pbig")
                    for half in range(2):
                        o = ps[:, 512 * half : 512 * (half + 1)]
                        for k in range(KT):
                            w_ap = (
                                wq(k)[:, P * m : P * (m + 1)]
                                if which == "q"
                                else wk_a[:, k, P * m : P * (m + 1)]
                            )
                            nc.tensor.matmul(
                                o,
                                lhsT=w_ap,
                                rhs=xt(k)[:, 512 * half : 512 * (half + 1)],
                                start=(k == 0),
                                stop=(k == KT - 1),
                            )
                    if biased:
                        nc.vector.tensor_scalar_add(dst, ps, bq_s[:, m : m + 1])
                    else:
                        nc.vector.tensor_copy(dst, ps)

            def proj_v(r0, r1):
                for r in range(r0, r1):
                    ps = psmall.tile([P, C], f32, name="pv", tag="sm")
                    for k in range(KT):
                        nc.tensor.matmul(
                            ps,
                            lhsT=xt(k)[:, P * r : P * (r + 1)],
                            rhs=wv_a[:, k, :],
                            start=(k == 0),
                            stop=(k == KT - 1),
                        )
                    nc.scalar.activation(v_t[r], ps, AF.Copy)

            es_t = {}

            def qk_phase(ic, p):
                for J in range(4 * (ic + 1)):
                    i0 = max(512 * ic, P * J)
                    w = 512 * (ic + 1) - i0
                    st = pbig.tile([P, 2, 512], f32, name="st", tag="pbig")
                    for h in range(2):
                        nc.tensor.matmul(
                            st[:, h, :w],
                            lhsT=kT_t[p][64 * h : 64 * (h + 1), P * J : P * (J + 1)],
                            rhs=qT_t[p][64 * h : 64 * (h + 1), i0 : i0 + w],
                            start=True,
                            stop=True,
                            tile_position=(64 * h, 0),
                        )
                    es = es_pool.tile([P, 2, 512], f16, name="es", tag="es")
                    es_t[(ic, p, J)] = es
                    nc.scalar.activation(
                        es[:, :, :w],
                        st[:, :, :w],
                        AF.Exp,
                        bias=cmask_s[:, J : J + 1],
                        scale=0.125,
                    )
                    if P * J >= 512 * ic:  # diagonal: zero the causal triangle
                        nc.vector.tensor_tensor(
                            out=es[:, :, :P], in0=es[:, :, :P], in1=tri_b, op=ALU.mult
                        )

            def av_phase(ic, p):
                av = psmall.tile([P, 512], f32, name="av", tag="sm")
                den = psmall.tile([P, 512], f32, name="den", tag="sm")
                nJ = 4 * (ic + 1)
                for J in range(nJ):
                    i0 = max(512 * ic, P * J)
                    w = 512 * (ic + 1) - i0
                    io = i0 - 512 * ic
                    first, last = J == 0, J == nJ - 1
                    es = es_t.pop((ic, p, J))
                    for h in range(2):
                        nc.tensor.matmul(
                            av[64 * h : 64 * (h + 1), io : io + w],
                            lhsT=v_t[J][:, P * p + 64 * h : P * p + 64 * (h + 1)],
                            rhs=es[:, h, :w],
                            start=first,
                            stop=last,
                            tile_position=(0, 64 * h),
                        )
                        nc.tensor.matmul(
                            den[64 * h : 64 * (h + 1), io : io + w],
                            lhsT=ones64_s,
                            rhs=es[:, h, :w],
                            start=first,
                            stop=last,
                            tile_position=(0, 64 * h),
                        )
                rden = rden_pool.tile([P, 512], f32, name="rden", tag="rden")
                nc.vector.reciprocal_approx_fast(out=rden, in_=den)
                nc.vector.tensor_mul(yn_t[p][:, 512 * ic : 512 * (ic + 1)], av, rden)

            def outproj(r0, r1):
                for r in range(r0, r1):
                    po = psmall.tile([P, C], f32, name="po", tag="sm")
                    for m in range(MT):
                        nc.tensor.matmul(
                            po,
                            lhsT=yn_t[m][:, P * r : P * (r + 1)],
                            rhs=wp_a[:, m, :],
                            start=(m == 0),
                            stop=(m == MT - 1),
                        )
                    ot = ot_pool.tile([P, C], f32, name="ot", tag="ot")
                    nc.vector.tensor_tensor(out=ot, in0=po, in1=bppb_s, op=ALU.add)
                    nc.sync.dma_start(out=out[P * r : P * (r + 1), :], in_=ot)

            # ---- emission schedule: weave attention into projections so the
            # ACT exp pipeline starts early and never starves ----
            proj_qk(0)
            qk_phase(0, 0)
            proj_qk(1)
            proj_v(0, 4)
            qk_phase(0, 1)
            av_phase(0, 0)
            proj_qk(2)
            proj_v(4, 8)
            qk_phase(0, 2)
            av_phase(0, 1)
            proj_qk(3)
            qk_phase(0, 3)
            av_phase(0, 2)
            av_phase(0, 3)
            qk_phase(1, 0)
            outproj(0, 4)
            qk_phase(1, 1)
            av_phase(1, 0)
            qk_phase(1, 2)
            av_phase(1, 1)
            qk_phase(1, 3)
            av_phase(1, 2)
            av_phase(1, 3)
            outproj(4, 8)

    nc.compile()
    return nc


def _prep_inputs(x, Wq, bq, Wk, bk, Wv, bv, Wp, bp):
    """Host-side prep: transposes, bias folding, mask tables. Returns in_maps."""
    f16 = np.float16
    wqT = np.ascontiguousarray(Wq.T).astype(f16)
    wkT = np.ascontiguousarray(Wk.T).astype(f16)
    wvT = np.ascontiguousarray(Wv.T).astype(f16)
    wpT = np.ascontiguousarray(Wp.T).astype(f16)
    bq_pp = np.ascontiguousarray(bq.astype(np.float32).reshape(C // P, P).T)
    # v bias folds into output bias: out = (y' + bv) @ Wp.T + bp
    bpp = (
        Wp.astype(np.float64) @ bv.astype(np.float64) + bp.astype(np.float64)
    ).astype(np.float32)
    bppb = np.broadcast_to(bpp[None, :], (P, C)).copy()
    ones64 = np.ones((P, HD), dtype=f16)
    tri = (np.arange(P)[:, None] <= np.arange(P)[None, :]).astype(f16)  # keep j<=i
    j_idx = np.arange(P)[:, None] + P * np.arange(T // P)[None, :]
    cmask = np.where(j_idx % JD == JD - 1, np.float32(NEG), np.float32(0.0)).astype(
        np.float32
    )

    shared = {
        "wqT": wqT,
        "wkT": wkT,
        "wvT": wvT,
        "wpT": wpT,
        "bq": bq_pp,
        "bppb": bppb,
        "ones64": ones64,
        "tri": tri,
        "cmask": cmask,
    }
    in_maps = []
    for b in range(N_CORES):
        m = dict(shared)
        m["xT"] = np.ascontiguousarray(x[b].T).astype(f16)
        in_maps.append(m)
    return in_maps


def kernel(x, Wq, bq, Wk, bk, Wv, bv, Wp, bp):
    from concourse import bass_utils

    x = np.asarray(x, dtype=np.float32)
    if "nc" not in _CACHE:
        _CACHE["nc"] = _build()
    nc = _CACHE["nc"]
    in_maps = _prep_inputs(
        x,
        np.asarray(Wq, np.float32),
        np.asarray(bq, np.float32),
        np.asarray(Wk, np.float32),
        np.asarray(bk, np.float32),
        np.asarray(Wv, np.float32),
        np.asarray(bv, np.float32),
        np.asarray(Wp, np.float32),
        np.asarray(bp, np.float32),
    )
    res = bass_utils.run_bass_kernel_spmd(nc, in_maps, core_ids=list(range(N_CORES)))
    return np.stack([res.results[b]["out"] for b in range(N_CORES)], axis=0)


# revision 14
# speedup vs baseline: 1.0400x; 1.0133x over previous
"""Causal self-attention (sparse column mask) on 8 Trainium2 NeuronCores.

Problem: B=8, T=1024, C=512, 8 heads (hd=64).
  q/k/v = x @ W{q,k,v}.T + b;  att = softmax(mask(q k^T / 8));  y = att v
  out = y @ Wp.T + bp
Mask: causal lower-triangle, minus every column j with j % 25 == 24.

Sharding: pure data-parallel over batch — core b computes batch element b.

Per-core kernel design (all matmul operands fp16, PSUM accumulation f32):
  - Host pre-transposes x[b] -> xT [C, T] and all weights -> W^T [c_in, c_out],
    so every on-chip matmul has its contraction dim on partitions.
  - Projections produce q^T, k^T [C, T] (heads = partition blocks of 64) and
    v [T, C]. q bias is added during PSUM evacuation (DVE tensor_scalar,
    per-partition); k bias is dropped (softmax shift invariance); v bias is
    folded into the output bias on host (bp' = Wp @ bv + bp, sent broadcast).
  - Attention, phase-separated per query chunk ic (512 wide) so the PE array
    stays in one tiling mode per phase (mode switches drain the PE):
      QK phase (64x128 row-tiled): per head-pair p, per key tile J: two K=64
      matmuls (tile_position (0,0)/(64,0)) -> S^T in 2 PSUM banks; one ACT
      exp call over both (scale=1/8, per-partition bias -30 on j%25==24
      columns) -> fp16 SBUF; causal diagonal zeroed by one fp16 multiply with
      a broadcast lower-triangle tile on DVE.
      AV phase (128x64 col-tiled): per pair, accumulate y'^T and the
      replicated denominators (ones-weight matmuls) over J; then rden via
      approx reciprocal and one tensor_tensor multiply PSUM->SBUF fp16.
  - Output projection consumes y_norm^T directly; bias added during the DVE
    evacuation (tensor_tensor add with a host-broadcast bias tile).
"""

import numpy as np

B, T, C = 8, 1024, 512
H = 8
HD = C // H
P = 128
JD = 25  # joined dim; column j masked when j % 25 == 24
N_CORES = 8
NEG = -30.0  # added post-scale; exp(-30) flushes to 0 in fp16

_CACHE = {}


def _build():
    import concourse.bass as bass
    import concourse.mybir as mybir
    import concourse.tile as tile
    from concourse import bacc

    f16 = mybir.dt.float16
    f32 = mybir.dt.float32
    AF = mybir.ActivationFunctionType
    ALU = mybir.AluOpType

    nc = bacc.Bacc("TRN2", target_bir_lowering=False, debug=False)

    xT = nc.dram_tensor("xT", [C, T], f16, kind="ExternalInput").ap()
    wqT = nc.dram_tensor("wqT", [C, C], f16, kind="ExternalInput").ap()
    wkT = nc.dram_tensor("wkT", [C, C], f16, kind="ExternalInput").ap()
    wvT = nc.dram_tensor("wvT", [C, C], f16, kind="ExternalInput").ap()
    wpT = nc.dram_tensor("wpT", [C, C], f16, kind="ExternalInput").ap()
    bq = nc.dram_tensor("bq", [P, C // P], f32, kind="ExternalInput").ap()
    bppb = nc.dram_tensor("bppb", [P, C], f32, kind="ExternalInput").ap()
    ones64 = nc.dram_tensor("ones64", [P, HD], f16, kind="ExternalInput").ap()
    tri = nc.dram_tensor("tri", [P, P], f16, kind="ExternalInput").ap()
    cmask = nc.dram_tensor("cmask", [P, T // P], f32, kind="ExternalInput").ap()
    out = nc.dram_tensor("out", [T, C], f32, kind="ExternalOutput").ap()

    KT = C // P  # 4 c_in tiles
    MT = C // P  # 4 c_out tiles (= head pairs)
    RT = T // P  # 8 t tiles

    with tile.TileContext(nc) as tc:
        with (
            tc.tile_pool(name="const", bufs=1) as const,
            tc.tile_pool(name="persist", bufs=1) as persist,
            tc.tile_pool(name="es", bufs=24) as es_pool,
            tc.tile_pool(name="rden", bufs=2) as rden_pool,
            tc.tile_pool(name="ot", bufs=3) as ot_pool,
            tc.tile_pool(name="pbig", bufs=2, space="PSUM") as pbig,
            tc.tile_pool(name="psmall", bufs=4, space="PSUM") as psmall,
        ):
            # ---- consolidated input loads (big DMAs, two queues) ----
            def load(shape, dtype, src, tag, eng):
                t = const.tile(shape, dtype, name=tag, tag=tag)
                eng.dma_start(out=t, in_=src)
                return t

            r3 = lambda a: a.rearrange("(a p) n -> p a n", p=P)  # noqa: E731
            # first-needed data in small chunks so the first matmuls start early
            xT_c = [
                load([P, 1, T], f16, r3(xT)[:, k : k + 1, :], f"xT{k}", nc.sync)
                for k in range(KT)
            ]
            wq_c = [
                load([P, 1, C], f16, r3(wqT)[:, k : k + 1, :], f"wqc{k}", nc.scalar)
                for k in range(KT)
            ]
            wk_a = load([P, KT, C], f16, r3(wkT), "wk", nc.scalar)
            tri_s = load([P, P], f16, tri, "tri", nc.sync)
            cmask_s = load([P, T // P], f32, cmask, "cmask", nc.sync)
            wv_a = load([P, KT, C], f16, r3(wvT), "wv", nc.sync)
            bq_s = load([P, C // P], f32, bq, "bq", nc.scalar)
            ones64_s = load([P, HD], f16, ones64, "ones64", nc.scalar)
            wp_a = load([P, KT, C], f16, r3(wpT), "wp", nc.scalar)
            bppb_s = load([P, C], f32, bppb, "bppb", nc.sync)

            def xt(k):
                return xT_c[k][:, 0, :]

            def wq(k):
                return wq_c[k][:, 0, :]

            qT_t = [persist.tile([P, T], f16, name=f"qT{m}", tag=f"qT{m}") for m in range(MT)]
            kT_t = [persist.tile([P, T], f16, name=f"kT{m}", tag=f"kT{m}") for m in range(MT)]
            v_t = [persist.tile([P, C], f16, name=f"v{r}", tag=f"v{r}") for r in range(RT)]
            yn_t = [persist.tile([P, T], f16, name=f"yn{m}", tag=f"yn{m}") for m in range(MT)]

            # broadcast lower-triangle tile across both heads of an es tile
            tri_b = bass.AP(
                tensor=tri_s.tensor,
                offset=tri_s.offset,
                ap=[list(tri_s.ap[0]), [0, 2], list(tri_s.ap[1])],
            )

            # ---- emission helpers ----
            def proj_qk(m):
                for which, dst, biased in (("q", qT_t[m], True), ("k", kT_t[m], False)):
                    ps = pbig.tile([P, T], f32, name="psqk", tag="pbig")
                    for half in range(2):
                        o = ps[:, 512 * half : 512 * (half + 1)]
                        for k in range(KT):
                            w_ap = (
                                wq(k)[:, P * m : P * (m + 1)]
                                if which == "q"
                                else wk_a[:, k, P * m : P * (m + 1)]
                            )
                            nc.tensor.matmul(
                                o,
                                lhsT=w_ap,
                                rhs=xt(k)[:, 512 * half : 512 * (half + 1)],
                                start=(k == 0),
                                stop=(k == KT - 1),
                            )
                    if biased:
                        nc.vector.tensor_scalar_add(dst, ps, bq_s[:, m : m + 1])
                    else:
                        nc.vector.tensor_copy(dst, ps)

            def proj_v(r0, r1):
                for r in range(r0, r1):
                    ps = psmall.tile([P, C], f32, name="pv", tag="sm")
                    for k in range(KT):
                        nc.tensor.matmul(
                            ps,
                            lhsT=xt(k)[:, P * r : P * (r + 1)],
                            rhs=wv_a[:, k, :],
                            start=(k == 0),
                            stop=(k == KT - 1),
                        )
                    nc.scalar.activation(v_t[r], ps, AF.Copy)

            es_t = {}

            def qk_phase(ic, p):
                for J in range(4 * (ic + 1)):
                    i0 = max(512 * ic, P * J)
                    w = 512 * (ic + 1) - i0
                    st = pbig.tile([P, 2, 512], f32, name="st", tag="pbig")
                    for h in range(2):
                        nc.tensor.matmul(
                            st[:, h, :w],
                            lhsT=kT_t[p][64 * h : 64 * (h + 1), P * J : P * (J + 1)],
                            rhs=qT_t[p][64 * h : 64 * (h + 1), i0 : i0 + w],
                            start=True,
                            stop=True,
                            tile_position=(64 * h, 0),
                        )
                    es = es_pool.tile([P, 2, 512], f16, name="es", tag="es")
                    es_t[(ic, p, J)] = es
                    nc.scalar.activation(
                        es[:, :, :w],
                        st[:, :, :w],
                        AF.Exp,
                        bias=cmask_s[:, J : J + 1],
                        scale=0.125,
                    )
                    if P * J >= 512 * ic:  # diagonal: zero the causal triangle
                        nc.vector.tensor_tensor(
                            out=es[:, :, :P], in0=es[:, :, :P], in1=tri_b, op=ALU.mult
                        )

            def av_phase(ic, p):
                av = psmall.tile([P, 512], f32, name="av", tag="sm")
                den = psmall.tile([P, 512], f32, name="den", tag="sm")
                nJ = 4 * (ic + 1)
                for J in range(nJ):
                    i0 = max(512 * ic, P * J)
                    w = 512 * (ic + 1) - i0
                    io = i0 - 512 * ic
                    first, last = J == 0, J == nJ - 1
                    es = es_t.pop((ic, p, J))
                    for h in range(2):
                        nc.tensor.matmul(
                            av[64 * h : 64 * (h + 1), io : io + w],
                            lhsT=v_t[J][:, P * p + 64 * h : P * p + 64 * (h + 1)],
                            rhs=es[:, h, :w],
                            start=first,
                            stop=last,
                            tile_position=(0, 64 * h),
                        )
                        nc.tensor.matmul(
                            den[64 * h : 64 * (h + 1), io : io + w],
                            lhsT=ones64_s,
                            rhs=es[:, h, :w],
                            start=first,
                            stop=last,
                            tile_position=(0, 64 * h),
                        )
                rden = rden_pool.tile([P, 512], f32, name="rden", tag="rden")
                nc.vector.reciprocal_approx_fast(out=rden, in_=den)
                nc.vector.tensor_mul(yn_t[p][:, 512 * ic : 512 * (ic + 1)], av, rden)

            def outproj(r0, r1):
                for r in range(r0, r1):
                    po = psmall.tile([P, C], f32, name="po", tag="sm")
                    for m in range(MT):
                        nc.tensor.matmul(
                            po,
                            lhsT=yn_t[m][:, P * r : P * (r + 1)],
                            rhs=wp_a[:, m, :],
                            start=(m == 0),
                            stop=(m == MT - 1),
                        )
                    ot = ot_pool.tile([P, C], f32, name="ot", tag="ot")
                    nc.vector.tensor_tensor(out=ot, in0=po, in1=bppb_s, op=ALU.add)
                    nc.sync.dma_start(out=out[P * r : P * (r + 1), :], in_=ot)

            # ---- emission schedule: weave attention into projections so the
            # ACT exp pipeline starts early and never starves ----
            proj_qk(0)
            qk_phase(0, 0)
            proj_qk(1)
            proj_v(0, 4)
            qk_phase(0, 1)
            av_phase(0, 0)
            proj_qk(2)
            proj_v(4, 8)
            qk_phase(0, 2)
            av_phase(0, 1)
            proj_qk(3)
            qk_phase(0, 3)
            av_phase(0, 2)
            av_phase(0, 3)
            qk_phase(1, 0)
            outproj(0, 4)
            qk_phase(1, 1)
            av_phase(1, 0)
            qk_phase(1, 2)
            av_phase(1, 1)
            qk_phase(1, 3)
            av_phase(1, 2)
            av_phase(1, 3)
            outproj(4, 8)

    nc.compile()
    return nc


def _prep_inputs(x, Wq, bq, Wk, bk, Wv, bv, Wp, bp):
    """Host-side prep: transposes, bias folding, mask tables. Returns in_maps."""
    f16 = np.float16
    wqT = np.ascontiguousarray(Wq.T).astype(f16)
    wkT = np.ascontiguousarray(Wk.T).astype(f16)
    wvT = np.ascontiguousarray(Wv.T).astype(f16)
    wpT = np.ascontiguousarray(Wp.T).astype(f16)
    bq_pp = np.ascontiguousarray(bq.astype(np.float32).reshape(C // P, P).T)
    # v bias folds into output bias: out = (y' + bv) @ Wp.T + bp
    bpp = (
        Wp.astype(np.float64) @ bv.astype(np.float64) + bp.astype(np.float64)
    ).astype(np.float32)
    bppb = np.broadcast_to(bpp[None, :], (P, C)).copy()
    ones64 = np.ones((P, HD), dtype=f16)
    tri = (np.arange(P)[:, None] <= np.arange(P)[None, :]).astype(f16)  # keep j<=i
    j_idx = np.arange(P)[:, None] + P * np.arange(T // P)[None, :]
    cmask = np.where(j_idx % JD == JD - 1, np.float32(NEG), np.float32(0.0)).astype(
        np.float32
    )

    shared = {
        "wqT": wqT,
        "wkT": wkT,
        "wvT": wvT,
        "wpT": wpT,
        "bq": bq_pp,
        "bppb": bppb,
        "ones64": ones64,
        "tri": tri,
        "cmask": cmask,
    }
    in_maps = []
    for b in range(N_CORES):
        m = dict(shared)
        m["xT"] = np.ascontiguousarray(x[b].T).astype(f16)
        in_maps.append(m)
    return in_maps


def kernel(x, Wq, bq, Wk, bk, Wv, bv, Wp, bp):
    from concourse import bass_utils

    x = np.asarray(x, dtype=np.float32)
    if "nc" not in _CACHE:
        _CACHE["nc"] = _build()
    nc = _CACHE["nc"]
    in_maps = _prep_inputs(
        x,
        np.asarray(Wq, np.float32),
        np.asarray(bq, np.float32),
        np.asarray(Wk, np.float32),
        np.asarray(bk, np.float32),
        np.asarray(Wv, np.float32),
        np.asarray(bv, np.float32),
        np.asarray(Wp, np.float32),
        np.asarray(bp, np.float32),
    )
    res = bass_utils.run_bass_kernel_spmd(nc, in_maps, core_ids=list(range(N_CORES)))
    return np.stack([res.results[b]["out"] for b in range(N_CORES)], axis=0)
